# revision 4
# baseline (speedup 1.0000x reference)
"""Causal BoW (running mean over T) Trainium2 kernel.

out[b, t, c] = sum_{s<=t} x[b, s, c] / (t+1)   for x of shape [32, 2048, 512] f32.

Sharding: batch B=32 across 8 NeuronCores (4 samples each), no cross-core comms.

Per-core algorithm (per sample [T=2048, C=512], 16 T-blocks of 128 rows):
  - Single-pass f32r matmuls: x is loaded into f32r tiles and streamed
    through the PE directly (1 cycle/column instead of 4 for f32). The
    ~11-bit mantissa truncation gives ~1e-4 relative output error, far
    inside the 2e-2 tolerance; no hi/lo split, no rounding copies.
  - Block offsets: one accumulating matmul group with "step" selector
    weights (step_k[p, m] = 1 if m > k) produces off[m, c] = sum_{k<m} tot_k
    in one PSUM bank.
  - Offset injection: off2[j] = x[b, j*128, :] + off[j] is one [16,512] DVE
    add (the block-top rows are pre-gathered from DRAM by a tiny DMA), then
    four small HWDGE SBUF->SBUF DMAs overwrite row 0 of each block with
    off2 (DMA moves data across partitions freely; compute engines cannot).
    Since every column m of U128 includes row 0, the scan matmul broadcasts
    the offset to all 128 output rows for free — this replaces a per-block
    K=1 broadcast matmul (~790 ns each on the PE). The scatters ride the SP
    ring, which is empty once the bulk loads have issued (~13 us); the
    SWDGE (gpsimd) ring was tried first and its single-partition-dst
    software DMAs took 8-20 us to complete, stalling every scan window.
  - Block scan: psum_j = U128^T.T @ x_j (U128 = upper-triangular ones).
  - Eviction: DVE copy with per-partition scale recip[p, j] = 1/(j*128+p+1)
    applied while moving PSUM -> SBUF.
  - x lives in quarter tiles [128, 4*512] (4 per sample): dependency
    tracking is tile-level, so quarter tiles let the first offset matmuls
    start ~3 us after launch instead of waiting a whole 4 MB sample load.
  - Fine-grained software pipeline: sample b+1's offset matmuls are
    interleaved between sample b's scan matmuls (slots 5..12, 2 per slot;
    off group closes at slot 13 so the off2-add + scatter chain hides
    behind the last scans). Evictions — and the output stream — never pause
    during offset bursts. All four samples' loads issue up front.
  - Input loads issue on the SP (sync) HW-DGE queue, output stores and
    constants on the Activation HW-DGE queue, so store issue never queues
    behind dependent load issue and both streams keep all 16 DMA engines
    fed. The last sample's output goes out in 2-block chunks to shorten the
    final drain.
"""

import numpy as np

import concourse.bass as bass
import concourse.bacc as bacc
import concourse.mybir as mybir
from concourse import tile
from concourse.bass_utils import run_bass_kernel_spmd

B, T, C = 32, 2048, 512
N_CORES = 8
BS = B // N_CORES          # samples per core
P = 128                    # partitions / T-block size
NBLK = T // P              # 16 blocks per sample
NQ = 4                     # quarters per sample
NH = NBLK // NQ            # blocks per quarter (4)
F32 = mybir.dt.float32
F32R = mybir.dt.float32r

_cache = {}


def _build():
    nc = bacc.Bacc()
    x = nc.dram_tensor("x", [BS, T, C], F32R, kind="ExternalInput")
    u128 = nc.dram_tensor("u128", [P, P], F32R, kind="ExternalInput")
    stepm = nc.dram_tensor("stepm", [P, NBLK * NBLK], F32R, kind="ExternalInput")
    recip = nc.dram_tensor("recip", [P, NBLK], F32, kind="ExternalInput")
    y = nc.dram_tensor("y", [BS, T, C], F32, kind="ExternalOutput")

    HALF = NH * C

    with tile.TileContext(nc) as tc:
        with (
            tc.tile_pool(name="singles", bufs=1) as singles,
            tc.tile_pool(name="xp", bufs=8) as xpool,
            tc.tile_pool(name="xrp", bufs=4) as xrpool,
            tc.tile_pool(name="op", bufs=4) as opool,
            tc.tile_pool(name="off2p", bufs=2) as off2pool,
            tc.tile_pool(name="pblk", bufs=6, space="PSUM") as pblk,
            tc.tile_pool(name="poff", bufs=2, space="PSUM") as poff,
        ):
            u_t = singles.tile([P, P], F32R)
            nc.scalar.dma_start(out=u_t[:], in_=u128[:])
            step_t = singles.tile([P, NBLK * NBLK], F32R)
            nc.scalar.dma_start(out=step_t[:], in_=stepm[:])
            recip_t = singles.tile([P, NBLK], F32)
            nc.scalar.dma_start(out=recip_t[:], in_=recip[:])

            def load(b):
                xs = x[b].rearrange("(j p) c -> p j c", p=P)   # [128, 16, 512]
                xts = []
                for h in range(NQ):
                    xt = xpool.tile([P, HALF], F32R, tag="xt", name="xt")
                    xt3 = xt.rearrange("p (j c) -> p j c", c=C)
                    nc.sync.dma_start(out=xt3[:],
                                      in_=xs[:, h * NH:(h + 1) * NH, :])
                    xts.append(xt)
                return xts

            def load_xr(b):
                # block-top rows x[b, j*128, :] as [16, 512] for off2
                xr = xrpool.tile([NBLK, C], F32R, tag="xr", name="xr")
                xj = x[b].rearrange("(j p) c -> j (p c)", p=P)
                nc.sync.dma_start(out=xr[:], in_=xj[:, 0:C])
                return xr

            def off_mm(xts, offp_t, k):
                sel = step_t[:, k * NBLK:(k + 1) * NBLK]
                nc.tensor.matmul(
                    offp_t[:], sel,
                    xts[k // NH][:, (k % NH) * C:(k % NH + 1) * C],
                    start=(k == 0), stop=(k == NBLK - 1),
                )

            def off_finish(xts, xr, offp_t):
                # off2[j] = x[b, j*128, :] + off[j]   (off[0] == 0)
                off2 = off2pool.tile([NBLK, C], F32R, tag="off2")
                nc.vector.tensor_add(out=off2[:], in0=offp_t[:], in1=xr[:])
                # overwrite row 0 of every block (partition 0 of each quarter).
                # These ride the SP (sync) HWDGE ring with the bulk loads.
                # HWDGE rings are FIFO, and the tile scheduler orders each
                # ring by sim-readiness, so with xpool bufs=8 (two samples in
                # flight) the ring comes out [L0, L1, c0, L2, c1, L3, c2, c3]:
                # each scatter drains right after the loads it needs, instead
                # of behind the whole 16.8 MB load stream (bufs=16 + sync) or
                # behind ~3 MB of store groups (Act ring), either of which
                # serializes the scan windows.
                for h in range(NQ):
                    nc.sync.dma_start(out=xts[h][0:1, :],
                                      in_=off2[h * NH:(h + 1) * NH, :])

            def scan_window(b, xts, nxt):
                ys = y[b].rearrange("(j p) c -> p j c", p=P)
                last = nxt is None
                if not last:
                    nxt_xts, nxt_xr = nxt
                    offp_t = poff.tile([NBLK, C], F32, tag="offp")
                ng, gb = (8, NH // 2) if last else (NQ, NH)
                for h in range(ng):
                    ot = opool.tile([P, gb * C], F32,
                                    tag="ot2" if last else "ot")
                    for jj in range(gb):
                        j = h * gb + jj
                        pb = pblk.tile([P, C], F32)
                        nc.tensor.matmul(
                            pb[:], u_t[:],
                            xts[j // NH][:, (j % NH) * C:(j % NH + 1) * C],
                            start=True, stop=True)
                        if not last and 5 <= j < 13:
                            off_mm(nxt_xts, offp_t, 2 * (j - 5))
                            off_mm(nxt_xts, offp_t, 2 * (j - 5) + 1)
                        elif not last and j == 13:
                            off_finish(nxt_xts, nxt_xr, offp_t)
                        nc.vector.tensor_scalar_mul(
                            ot[:, jj * C:(jj + 1) * C], pb[:],
                            recip_t[:, j:j + 1]
                        )
                    ot3 = ot.rearrange("p (j c) -> p j c", c=C)
                    nc.scalar.dma_start(
                        out=ys[:, h * gb:(h + 1) * gb, :], in_=ot3[:]
                    )

            # prologue: all loads up front; sample 0's offsets + injection
            xts = [load(0)]
            xrs = [load_xr(0)]
            for bb in range(1, BS):
                xts.append(load(bb))
                xrs.append(load_xr(bb))
            offp0 = poff.tile([NBLK, C], F32, tag="offp")
            for k in range(NBLK):
                off_mm(xts[0], offp0, k)
            off_finish(xts[0], xrs[0], offp0)

            for b in range(BS):
                nxt = (xts[b + 1], xrs[b + 1]) if b + 1 < BS else None
                scan_window(b, xts[b], nxt)
    nc.finalize()
    return nc


def _consts():
    u = np.triu(np.ones((P, P), dtype=np.float32))
    step = np.zeros((P, NBLK * NBLK), dtype=np.float32)
    for k in range(NBLK):
        for m in range(NBLK):
            if m > k:
                step[:, k * NBLK + m] = 1.0
    recip = (1.0 / np.arange(1, T + 1, dtype=np.float32)).reshape(NBLK, P).T.copy()
    return u, step, recip


def run(x, trace=False):
    x = np.ascontiguousarray(np.asarray(x, dtype=np.float32))
    assert x.shape == (B, T, C), x.shape
    if "nc" not in _cache:
        _cache["nc"] = _build()
    nc = _cache["nc"]
    u, step, recip = _consts()
    in_maps = [
        {
            "x": np.ascontiguousarray(x[i * BS:(i + 1) * BS]),
            "u128": u,
            "stepm": step,
            "recip": recip,
        }
        for i in range(N_CORES)
    ]
    res = run_bass_kernel_spmd(nc, in_maps, list(range(N_CORES)), trace=trace)
    y = np.concatenate([res.results[i]["y"] for i in range(N_CORES)], axis=0)
    return y, res.exec_time_ns


def kernel(x):
    y, _ = run(x, trace=False)
    return y



# revision 11
# speedup vs baseline: 1.1828x; 1.1828x over previous
"""Causal BoW (running mean over T) Trainium2 kernel.

out[b, t, c] = sum_{s<=t} x[b, s, c] / (t+1)   for x of shape [32, 2048, 512] f32.

Sharding: batch B=32 across 8 NeuronCores (4 samples each), no cross-core comms.

Per-core algorithm (per sample [T=2048, C=512], 16 T-blocks of 128 rows):
  - Single-pass f32r matmuls: x is loaded into f32r tiles and streamed
    through the PE directly (1 cycle/column instead of 4 for f32). The
    ~11-bit mantissa truncation gives ~1e-4 relative output error, far
    inside the 2e-2 tolerance.
  - Block offsets: one accumulating matmul group with "step" selector
    weights (step_k[p, m] = 1 if m > k) produces off[m, c] = sum_{k<m} tot_k
    in one PSUM bank; a scalar-engine copy evicts it to SBUF (off2).
  - Offset injection happens ENTIRELY ON THE PE: each block's PSUM bank is
    an accumulation group of two matmuls, psum_j = U128^T @ x_j (start)
    then += ones[1,128]^T @ off2[j] (K=1 broadcast, stop). Block 0 needs no
    offset. This keeps every cross-engine DMA (row-0 scatter via SBUF->SBUF
    DMA in earlier versions) out of the offs->scans dependency chain: HWDGE
    rings are FIFO, so a tiny scatter queued behind bulk loads (SP ring) or
    behind store groups (Act ring) adds 10-40 us of serialization per scan
    window. The K=1 matmuls cost ~215-430 ns each and ride the PE, which
    has slack once HAM un-throttles (1.2 -> 2.4 GHz under sustained load).
  - Block scan: psum_j = U128^T.T @ x_j (U128 = upper-triangular ones).
  - Eviction: DVE copy with per-partition scale recip[p, j] = 1/(j*128+p+1)
    applied while moving PSUM -> SBUF. Block 0's eviction only waits on its
    single matmul, so the store stream starts ~10 us earlier than with
    row-0 injection (which gated every scan on the injected offset).
  - x lives in quarter tiles [128, 4*512] (4 per sample, 16 bufs = all four
    samples resident): tile-level dependency tracking lets the first offset
    matmuls start ~3 us after launch, and keeping all loads up front leaves
    the SP ring a pure load stream that drains at HBM read rate.
  - Software pipeline: sample b+1's offset matmuls are interleaved between
    sample b's scan matmuls (slots 5..12, 2 per slot; the off group closes
    at slot 13 so the scalar-copy hides behind the last scans).
  - Input loads issue on the SP (sync) HW-DGE queue, output stores and
    constants on the Activation HW-DGE queue, so store issue never queues
    behind dependent load issue and both streams keep all 16 DMA engines
    fed. The last sample's output goes out in 2-block chunks to shorten the
    final drain.
"""

import numpy as np

import concourse.bass as bass
import concourse.bacc as bacc
import concourse.mybir as mybir
from concourse import tile
from concourse.bass_utils import run_bass_kernel_spmd

B, T, C = 32, 2048, 512
N_CORES = 8
BS = B // N_CORES          # samples per core
P = 128                    # partitions / T-block size
NBLK = T // P              # 16 blocks per sample
NQ = 4                     # quarters per sample
NH = NBLK // NQ            # blocks per quarter (4)
F32 = mybir.dt.float32
F32R = mybir.dt.float32r

_cache = {}


def _build():
    nc = bacc.Bacc()
    x = nc.dram_tensor("x", [BS, T, C], F32R, kind="ExternalInput")
    u128 = nc.dram_tensor("u128", [P, P], F32R, kind="ExternalInput")
    stepm = nc.dram_tensor("stepm", [P, NBLK * NBLK], F32R, kind="ExternalInput")
    recip = nc.dram_tensor("recip", [P, NBLK], F32, kind="ExternalInput")
    sel16 = nc.dram_tensor("sel16", [NBLK, NBLK * P], F32R, kind="ExternalInput")
    y = nc.dram_tensor("y", [BS, T, C], F32, kind="ExternalOutput")

    HALF = NH * C

    with tile.TileContext(nc) as tc:
        with (
            tc.tile_pool(name="singles", bufs=1) as singles,
            tc.tile_pool(name="xp", bufs=16) as xpool,
            tc.tile_pool(name="op", bufs=4) as opool,
            tc.tile_pool(name="off2p", bufs=2) as off2pool,
            tc.tile_pool(name="pblk", bufs=6, space="PSUM") as pblk,
            tc.tile_pool(name="poff", bufs=2, space="PSUM") as poff,
        ):
            u_t = singles.tile([P, P], F32R)
            nc.scalar.dma_start(out=u_t[:], in_=u128[:])
            step_t = singles.tile([P, NBLK * NBLK], F32R)
            nc.scalar.dma_start(out=step_t[:], in_=stepm[:])
            recip_t = singles.tile([P, NBLK], F32)
            nc.scalar.dma_start(out=recip_t[:], in_=recip[:])
            # one-hot row selectors: sel16[k, j*128+p] = (k == j). A K=16
            # matmul with lhsT = sel16[:, j*128:(j+1)*128] broadcasts row j
            # of the off2 tile to all 128 output partitions (PE rhs base
            # partition must be 0/32/64, so a [1,512] row-j rhs is illegal).
            sel16_t = singles.tile([NBLK, NBLK * P], F32R)
            nc.scalar.dma_start(out=sel16_t[:], in_=sel16[:])

            def load(b):
                xs = x[b].rearrange("(j p) c -> p j c", p=P)   # [128, 16, 512]
                xts = []
                for h in range(NQ):
                    xt = xpool.tile([P, HALF], F32R, tag="xt", name="xt")
                    xt3 = xt.rearrange("p (j c) -> p j c", c=C)
                    nc.sync.dma_start(out=xt3[:],
                                      in_=xs[:, h * NH:(h + 1) * NH, :])
                    xts.append(xt)
                return xts

            def off_mm(xts, offp_t, k):
                sel = step_t[:, k * NBLK:(k + 1) * NBLK]
                nc.tensor.matmul(
                    offp_t[:], sel,
                    xts[k // NH][:, (k % NH) * C:(k % NH + 1) * C],
                    start=(k == 0), stop=(k == NBLK - 1),
                )

            def off_finish(offp_t):
                # evict the offset bank to SBUF so the K=1 broadcast matmuls
                # can stream it (matmul rhs must be SBUF)
                off2 = off2pool.tile([NBLK, C], F32R, tag="off2")
                nc.scalar.copy(out=off2[:], in_=offp_t[:])
                return off2

            def scan_window(b, xts, off2b, nxt):
                ys = y[b].rearrange("(j p) c -> p j c", p=P)
                last = nxt is None
                if not last:
                    offp_t = poff.tile([NBLK, C], F32, tag="offp")
                off2_next = None
                ng, gb = (8, NH // 2) if last else (NQ, NH)
                for h in range(ng):
                    ot = opool.tile([P, gb * C], F32,
                                    tag="ot2" if last else "ot")
                    for jj in range(gb):
                        j = h * gb + jj
                        pb = pblk.tile([P, C], F32)
                        nc.tensor.matmul(
                            pb[:], u_t[:],
                            xts[j // NH][:, (j % NH) * C:(j % NH + 1) * C],
                            start=True, stop=(j == 0))
                        if j > 0:
                            nc.tensor.matmul(
                                pb[:], sel16_t[:, j * P:(j + 1) * P],
                                off2b[:], start=False, stop=True)
                        if not last and 5 <= j < 13:
                            off_mm(nxt, offp_t, 2 * (j - 5))
                            off_mm(nxt, offp_t, 2 * (j - 5) + 1)
                        elif not last and j == 13:
                            off2_next = off_finish(offp_t)
                        nc.vector.tensor_scalar_mul(
                            ot[:, jj * C:(jj + 1) * C], pb[:],
                            recip_t[:, j:j + 1]
                        )
                    ot3 = ot.rearrange("p (j c) -> p j c", c=C)
                    nc.scalar.dma_start(
                        out=ys[:, h * gb:(h + 1) * gb, :], in_=ot3[:]
                    )
                return off2_next

            # prologue: all loads up front; sample 0's offset group
            xts = [load(b) for b in range(BS)]
            offp0 = poff.tile([NBLK, C], F32, tag="offp")
            for k in range(NBLK):
                off_mm(xts[0], offp0, k)
            off2_cur = off_finish(offp0)

            for b in range(BS):
                nxt = xts[b + 1] if b + 1 < BS else None
                off2_cur = scan_window(b, xts[b], off2_cur, nxt)
    nc.finalize()
    return nc


def _consts():
    u = np.triu(np.ones((P, P), dtype=np.float32))
    step = np.zeros((P, NBLK * NBLK), dtype=np.float32)
    for k in range(NBLK):
        for m in range(NBLK):
            if m > k:
                step[:, k * NBLK + m] = 1.0
    recip = (1.0 / np.arange(1, T + 1, dtype=np.float32)).reshape(NBLK, P).T.copy()
    sel = np.zeros((NBLK, NBLK * P), dtype=np.float32)
    for j in range(NBLK):
        sel[j, j * P:(j + 1) * P] = 1.0
    return u, step, recip, sel


def run(x, trace=False):
    x = np.ascontiguousarray(np.asarray(x, dtype=np.float32))
    assert x.shape == (B, T, C), x.shape
    if "nc" not in _cache:
        _cache["nc"] = _build()
    nc = _cache["nc"]
    u, step, recip, sel = _consts()
    in_maps = [
        {
            "x": np.ascontiguousarray(x[i * BS:(i + 1) * BS]),
            "u128": u,
            "stepm": step,
            "recip": recip,
            "sel16": sel,
        }
        for i in range(N_CORES)
    ]
    res = run_bass_kernel_spmd(nc, in_maps, list(range(N_CORES)), trace=trace)
    y = np.concatenate([res.results[i]["y"] for i in range(N_CORES)], axis=0)
    return y, res.exec_time_ns


def kernel(x):
    y, _ = run(x, trace=False)
    return y
